# revision 9
# baseline (speedup 1.0000x reference)
"""Graphormer multi-head attention on 8 Trainium2 cores.

Sharding: 2 cores per batch element (B=4), each core handling 8 of 16 heads
(tensor-parallel within the batch). Per core:
  - QKV projections for its 512 local feature columns (transposed layouts)
  - scoresT[s,t] = K_h Q_h^T per head (K=64 contraction on PE)
  - p = exp(scoresT) * expbT  (expbT = exp(attn_mask + edge_bias).T from host;
    no max-subtraction needed: |scores| stays < ~8)
  - PV with a ones-column appended to V -> row 64 of PSUM = softmax denom
  - normalize via reciprocal + partition-broadcast, out-project 512 local
    features into a full [E,T] partial, summed with the pair core on host.
PV(h-1) matmuls are interleaved into QK(h)'s loop to keep the PE dense
(HAM stays un-throttled). All matmuls run bf16 with fp32 PSUM accumulation.
"""
import sys

sys.path.insert(0, '/opt/trn_rl_repo')

import ml_dtypes
import numpy as np

import concourse.bass as bass
import concourse.mybir as mybir
import concourse.tile as tile
from concourse import bacc
from concourse.bass_utils import run_bass_kernel_spmd

DT = mybir.dt

B, T, S, E, H = 4, 1024, 1024, 1024, 16
D = E // H          # 64
HL = 8              # heads per core
F = HL * D          # 512 local features
N_CORES = 8

MMDT = DT.bfloat16          # matmul operand dtype
NP_MMDT = ml_dtypes.bfloat16


def _build_program():
    nc = bacc.Bacc()

    xqT = nc.dram_tensor("xqT", [E, T], MMDT, kind="ExternalInput")
    xkT = nc.dram_tensor("xkT", [E, S], MMDT, kind="ExternalInput")
    xvT = nc.dram_tensor("xvT", [E, S], MMDT, kind="ExternalInput")
    wqT = nc.dram_tensor("wqT", [E, F], MMDT, kind="ExternalInput")
    wkT = nc.dram_tensor("wkT", [E, F], MMDT, kind="ExternalInput")
    wvT = nc.dram_tensor("wvT", [E, F], MMDT, kind="ExternalInput")
    woT = nc.dram_tensor("woT", [F, E], MMDT, kind="ExternalInput")
    bq = nc.dram_tensor("bq", [128, 4], DT.float32, kind="ExternalInput")
    bk = nc.dram_tensor("bk", [128, 4], DT.float32, kind="ExternalInput")
    bv = nc.dram_tensor("bv", [1, F], DT.float32, kind="ExternalInput")
    expbT = nc.dram_tensor("expbT", [S, T], MMDT, kind="ExternalInput")
    yT = nc.dram_tensor("yT", [E, T], DT.float32, kind="ExternalOutput")
    yTa = nc.dram_tensor("yTa", [E, T], DT.float32, kind="ExternalOutput")

    with tile.TileContext(nc) as tc:
        with tc.tile_pool(name="persist", bufs=1) as pp, \
             tc.tile_pool(name="xin", bufs=2) as xp, \
             tc.tile_pool(name="work", bufs=3) as wk, \
             tc.tile_pool(name="pT", bufs=2) as pTp, \
             tc.tile_pool(name="ps", bufs=3, space="PSUM") as ps, \
             tc.tile_pool(name="pspv", bufs=1, space="PSUM") as pspv:

            # ---- q projection inputs first (critical path) ----
            wq_sb = pp.tile([128, 8, F], MMDT, tag="wq")
            nc.sync.dma_start(wq_sb[:], wqT[:].rearrange("(c p) f -> p c f", p=128))
            bq_sb = pp.tile([128, 4], DT.float32, tag="bq")
            nc.sync.dma_start(bq_sb[:], bq[:])
            qT_sb = pp.tile([128, 4, T], MMDT, tag="qT")
            kT_sb = pp.tile([128, 4, S], MMDT, tag="kT")
            xq_sb = xp.tile([128, 8, T], MMDT, tag="x")
            xqT_v = xqT[:].rearrange("(c p) t -> p c t", p=128)
            for ec in range(8):
                nc.sync.dma_start(xq_sb[:, ec, :], xqT_v[:, ec, :])

            wk_sb = pp.tile([128, 8, F], MMDT, tag="wk")
            nc.sync.dma_start(wk_sb[:], wkT[:].rearrange("(c p) f -> p c f", p=128))
            bk_sb = pp.tile([128, 4], DT.float32, tag="bk")
            nc.sync.dma_start(bk_sb[:], bk[:])
            xk_sb = xp.tile([128, 8, S], MMDT, tag="x")
            nc.sync.dma_start(xk_sb[:], xkT[:].rearrange("(c p) t -> p c t", p=128))

            for x_sb, w_sb, b_sb, dst in (
                (xq_sb, wq_sb, bq_sb, qT_sb),
                (xk_sb, wk_sb, bk_sb, kT_sb),
            ):
                for fc in range(4):
                    acc = ps.tile([128, T], DT.float32, tag="mm")
                    for th in range(2):
                        for ec in range(8):
                            nc.tensor.matmul(
                                acc[:, th * 512:(th + 1) * 512],
                                w_sb[:, ec, fc * 128:(fc + 1) * 128],
                                x_sb[:, ec, th * 512:(th + 1) * 512],
                                start=(ec == 0), stop=(ec == 7),
                            )
                    nc.scalar.add(dst[:, fc, :], acc[:], b_sb[:, fc:fc + 1])

            # ---- v projection into ones-augmented layout [s, h*65+d] ----
            wv_sb = pp.tile([128, 8, F], MMDT, tag="wv")
            nc.sync.dma_start(wv_sb[:], wvT[:].rearrange("(c p) f -> p c f", p=128))
            bv_row = pp.tile([1, F], DT.float32, tag="bvrow")
            nc.sync.dma_start(bv_row[:], bv[:])
            bv_bc = pp.tile([128, F], DT.float32, tag="bvbc")
            nc.gpsimd.partition_broadcast(bv_bc[:], bv_row[:])
            xv_sb = xp.tile([128, 8, S], MMDT, tag="x")
            nc.sync.dma_start(xv_sb[:], xvT[:].rearrange("(c p) s -> p c s", p=128))
            v_sb = pp.tile([128, 8, HL * 65], MMDT, tag="v")
            for sc in range(8):
                acc = ps.tile([128, F], DT.float32, tag="mm")
                for ec in range(8):
                    nc.tensor.matmul(
                        acc[:],
                        xv_sb[:, ec, sc * 128:(sc + 1) * 128],
                        wv_sb[:, ec, :],
                        start=(ec == 0), stop=(ec == 7),
                    )
                vv = v_sb[:, sc, :].rearrange("p (h c) -> p h c", c=65)
                nc.vector.tensor_add(
                    vv[:, :, 0:64],
                    acc[:].rearrange("p (h d) -> p h d", d=64),
                    bv_bc[:].rearrange("p (h d) -> p h d", d=64),
                )
                nc.vector.memset(vv[:, :, 64:65], 1.0)

            # ---- exp(biasT) from host ----
            expb_sb = pp.tile([128, 8, T], MMDT, tag="expb")
            nc.sync.dma_start(expb_sb[:], expbT[:].rearrange("(c p) t -> p c t", p=128))

            wo_sb = pp.tile([128, 4, E], MMDT, tag="wo")
            nc.sync.dma_start(wo_sb[:], woT[:].rearrange("(c p) e -> p c e", p=128))

            # ---- attention: QK(h) interleaved with PV(h-1) MMs ----
            oT_sb = pp.tile([128, 4, T], MMDT, tag="oT")
            state = {}

            def emit_qk_chunk(h, sc):
                hc, po = h // 2, 64 * (h % 2)
                if sc == 0:
                    pT_new = pTp.tile([128, 8, T], MMDT, tag="pT")
                    state[h] = pT_new
                pT = state[h]
                sps = ps.tile([128, T], DT.float32, tag="mm")
                for th in range(2):
                    nc.tensor.matmul(
                        sps[:, th * 512:(th + 1) * 512],
                        kT_sb[po:po + 64, hc, sc * 128:(sc + 1) * 128],
                        qT_sb[po:po + 64, hc, th * 512:(th + 1) * 512],
                        start=True, stop=True,
                    )
                et = wk.tile([128, T], MMDT, tag="exps")
                nc.scalar.activation(et[:], sps[:],
                                     mybir.ActivationFunctionType.Exp)
                nc.vector.tensor_mul(pT[:, sc, :], et[:], expb_sb[:, sc, :])

            def emit_pv_chunk(h, i):
                # i in 0..7 -> PV matmuls 2i, 2i+1 of head h (16 total)
                pT = state[h]
                if i == 0:
                    ops_new = pspv.tile([65, T], DT.float32, tag="pv")
                    state[(h, "ops")] = ops_new
                ops = state[(h, "ops")]
                for j in (2 * i, 2 * i + 1):
                    th, sc = divmod(j, 8)
                    nc.tensor.matmul(
                        ops[:, th * 512:(th + 1) * 512],
                        v_sb[:, sc, h * 65:(h + 1) * 65],
                        pT[:, sc, th * 512:(th + 1) * 512],
                        start=(sc == 0), stop=(sc == 7),
                    )

            def emit_norm(h):
                hc, po = h // 2, 64 * (h % 2)
                ops = state.pop((h, "ops"))
                del state[h]
                lrow = wk.tile([1, T], DT.float32, tag="lrow")
                nc.vector.tensor_copy(lrow[:], ops[64:65, :])
                rl = wk.tile([1, T], DT.float32, tag="rl")
                nc.vector.reciprocal_approx_fast(out=rl[:], in_=lrow[:])
                rlb = wk.tile([64, T], DT.float32, tag="rlb")
                nc.gpsimd.partition_broadcast(rlb[:], rl[:])
                nc.vector.tensor_mul(oT_sb[po:po + 64, hc, :], ops[0:64, :], rlb[:])

            # heads-0..3 half of the output projection, run during heads 5-6
            # and shipped out as a separate partial (host sums the halves).
            def emit_y01_chunk(slot):
                # slot 0..15 -> (ec8, th) pair; 2 MMs (fc 0,1) + copy on th=1
                ec8, th = divmod(slot, 2)
                if th == 0:
                    yps_new = ps.tile([128, T], DT.float32, tag="mm")
                    state[("y", ec8)] = yps_new
                yps = state[("y", ec8)]
                for fc in range(2):
                    nc.tensor.matmul(
                        yps[:, th * 512:(th + 1) * 512],
                        wo_sb[:, fc, ec8 * 128:(ec8 + 1) * 128],
                        oT_sb[:, fc, th * 512:(th + 1) * 512],
                        start=(fc == 0), stop=(fc == 1),
                    )
                if th == 1:
                    ya = wk.tile([128, T], DT.float32, tag="yout")
                    nc.vector.tensor_copy(ya[:], state.pop(("y", ec8))[:])
                    nc.sync.dma_start(yTa[ec8 * 128:(ec8 + 1) * 128, :], ya[:])

            for sc in range(8):
                emit_qk_chunk(0, sc)
            for h in range(1, HL):
                for sc in range(8):
                    emit_qk_chunk(h, sc)
                    emit_pv_chunk(h - 1, sc)
                    if h in (5, 6):
                        emit_y01_chunk((h - 5) * 8 + sc)
                emit_norm(h - 1)
            for i in range(8):
                emit_pv_chunk(HL - 1, i)
            emit_norm(HL - 1)

            # ---- tail: heads 4..7 half of the output projection ----
            for ec8 in range(8):
                yps = ps.tile([128, T], DT.float32, tag="mm")
                for th in range(2):
                    for fc in range(2, 4):
                        nc.tensor.matmul(
                            yps[:, th * 512:(th + 1) * 512],
                            wo_sb[:, fc, ec8 * 128:(ec8 + 1) * 128],
                            oT_sb[:, fc, th * 512:(th + 1) * 512],
                            start=(fc == 2), stop=(fc == 3),
                        )
                yout = wk.tile([128, T], DT.float32, tag="yout")
                nc.vector.tensor_copy(yout[:], yps[:])
                nc.sync.dma_start(yT[ec8 * 128:(ec8 + 1) * 128, :], yout[:])

    nc.compile()
    return nc


_NC_CACHE = []


def kernel(query, key_, value, edge_bias, attn_mask, key_padding_mask,
           Wq, bq, Wk, bk, Wv, bv, Wo, bo):
    if not _NC_CACHE:
        _NC_CACHE.append(_build_program())
    nc = _NC_CACHE[0]

    scale = np.float32(D ** -0.5)
    q32, k32, v32 = (np.asarray(a, np.float32) for a in (query, key_, value))
    WqT = (np.asarray(Wq, np.float32).T * scale).astype(NP_MMDT)
    WkT = np.asarray(Wk, np.float32).T.astype(NP_MMDT)
    WvT = np.asarray(Wv, np.float32).T.astype(NP_MMDT)
    WoT = np.asarray(Wo, np.float32).T
    bq_s = (np.asarray(bq, np.float32) * scale)
    kpm_add = np.where(np.asarray(key_padding_mask), np.float32(-1e30),
                       np.float32(0.0))  # [B, S]
    mask32 = np.asarray(attn_mask, np.float32)

    in_maps = []
    for c in range(N_CORES):
        b, g = divmod(c, 2)
        cols = slice(g * F, (g + 1) * F)
        bias_sb = (mask32 + np.asarray(edge_bias[b], np.float32)
                   + kpm_add[b][None, :])  # [T, S]
        in_maps.append({
            "xqT": np.ascontiguousarray(q32[b].T).astype(NP_MMDT),
            "xkT": np.ascontiguousarray(k32[b].T).astype(NP_MMDT),
            "xvT": np.ascontiguousarray(v32[b].T).astype(NP_MMDT),
            "wqT": np.ascontiguousarray(WqT[:, cols]),
            "wkT": np.ascontiguousarray(WkT[:, cols]),
            "wvT": np.ascontiguousarray(WvT[:, cols]),
            "woT": np.ascontiguousarray(WoT[cols, :]).astype(NP_MMDT),
            "bq": np.ascontiguousarray(bq_s[cols].reshape(4, 128).T),
            "bk": np.ascontiguousarray(np.asarray(bk, np.float32)[cols]
                                       .reshape(4, 128).T),
            "bv": np.asarray(bv, np.float32)[cols].reshape(1, F),
            "expbT": np.exp(bias_sb.T).astype(NP_MMDT),
        })

    res = run_bass_kernel_spmd(nc, in_maps, list(range(N_CORES)))

    out = np.empty((B, T, E), np.float32)
    bo32 = np.asarray(bo, np.float32)
    for b in range(B):
        r0, r1 = res.results[2 * b], res.results[2 * b + 1]
        acc = r0["yT"] + r0["yTa"] + r1["yT"] + r1["yTa"]
        out[b] = acc.T + bo32[None, :]
    return out


# revision 10
# speedup vs baseline: 1.1104x; 1.1104x over previous
"""Graphormer multi-head attention on 8 Trainium2 cores.

Sharding: 2 cores per batch element (B=4), each core handling 8 of 16 heads
(tensor-parallel within the batch). Per core:
  - QKV projections for its 512 local feature columns (transposed layouts)
  - scoresT[s,t] = K_h Q_h^T per head (K=64 contraction on PE)
  - p = exp(scoresT) * expbT  (expbT = exp(attn_mask + edge_bias).T from host;
    no max-subtraction needed: |scores| stays < ~8)
  - PV with a ones-column appended to V -> row 64 of PSUM = softmax denom
  - normalize via reciprocal + partition-broadcast, out-project 512 local
    features into a full [E,T] partial, summed with the pair core on host.
PV(h-1) matmuls are interleaved into QK(h)'s loop to keep the PE dense
(HAM stays un-throttled). All matmuls run bf16 with fp32 PSUM accumulation.
"""
import sys

sys.path.insert(0, '/opt/trn_rl_repo')

import ml_dtypes
import numpy as np

import concourse.bass as bass
import concourse.mybir as mybir
import concourse.tile as tile
from concourse import bacc
from concourse.bass_utils import run_bass_kernel_spmd

DT = mybir.dt

B, T, S, E, H = 4, 1024, 1024, 1024, 16
D = E // H          # 64
HL = 8              # heads per core
F = HL * D          # 512 local features
N_CORES = 8

MMDT = DT.bfloat16          # matmul operand dtype
NP_MMDT = ml_dtypes.bfloat16


def _build_program():
    nc = bacc.Bacc()

    xqT = nc.dram_tensor("xqT", [E, T], MMDT, kind="ExternalInput")
    xkT = nc.dram_tensor("xkT", [E, S], MMDT, kind="ExternalInput")
    xvT = nc.dram_tensor("xvT", [E, S], MMDT, kind="ExternalInput")
    wqT = nc.dram_tensor("wqT", [E, F], MMDT, kind="ExternalInput")
    wkT = nc.dram_tensor("wkT", [E, F], MMDT, kind="ExternalInput")
    wvT = nc.dram_tensor("wvT", [E, F], MMDT, kind="ExternalInput")
    woT = nc.dram_tensor("woT", [F, E], MMDT, kind="ExternalInput")
    bq = nc.dram_tensor("bq", [128, 4], DT.float32, kind="ExternalInput")
    bk = nc.dram_tensor("bk", [128, 4], DT.float32, kind="ExternalInput")
    bv = nc.dram_tensor("bv", [1, F], DT.float32, kind="ExternalInput")
    expbT = nc.dram_tensor("expbT", [S, T], MMDT, kind="ExternalInput")
    yT = nc.dram_tensor("yT", [E, T], DT.float32, kind="ExternalOutput")
    yTa = nc.dram_tensor("yTa", [E, T], DT.float32, kind="ExternalOutput")

    with tile.TileContext(nc) as tc:
        with tc.tile_pool(name="persist", bufs=1) as pp, \
             tc.tile_pool(name="xin", bufs=2) as xp, \
             tc.tile_pool(name="work", bufs=3) as wk, \
             tc.tile_pool(name="pT", bufs=2) as pTp, \
             tc.tile_pool(name="ps", bufs=2, space="PSUM") as ps, \
             tc.tile_pool(name="pspv", bufs=2, space="PSUM") as pspv:

            # ---- q projection inputs first (critical path) ----
            wq_sb = pp.tile([128, 8, F], MMDT, tag="wq")
            nc.sync.dma_start(wq_sb[:], wqT[:].rearrange("(c p) f -> p c f", p=128))
            bq_sb = pp.tile([128, 4], DT.float32, tag="bq")
            nc.sync.dma_start(bq_sb[:], bq[:])
            qT_sb = pp.tile([128, 4, T], MMDT, tag="qT")
            kT_sb = pp.tile([128, 4, S], MMDT, tag="kT")
            xq_sb = xp.tile([128, 8, T], MMDT, tag="x")
            xqT_v = xqT[:].rearrange("(c p) t -> p c t", p=128)
            for ec in range(8):
                nc.sync.dma_start(xq_sb[:, ec, :], xqT_v[:, ec, :])

            wk_sb = pp.tile([128, 8, F], MMDT, tag="wk")
            nc.sync.dma_start(wk_sb[:], wkT[:].rearrange("(c p) f -> p c f", p=128))
            bk_sb = pp.tile([128, 4], DT.float32, tag="bk")
            nc.sync.dma_start(bk_sb[:], bk[:])
            xk_sb = xp.tile([128, 8, S], MMDT, tag="x")
            nc.sync.dma_start(xk_sb[:], xkT[:].rearrange("(c p) t -> p c t", p=128))

            for x_sb, w_sb, b_sb, dst in (
                (xq_sb, wq_sb, bq_sb, qT_sb),
                (xk_sb, wk_sb, bk_sb, kT_sb),
            ):
                for fc in range(4):
                    acc = ps.tile([128, T], DT.float32, tag="mm")
                    for th in range(2):
                        for ec in range(8):
                            nc.tensor.matmul(
                                acc[:, th * 512:(th + 1) * 512],
                                w_sb[:, ec, fc * 128:(fc + 1) * 128],
                                x_sb[:, ec, th * 512:(th + 1) * 512],
                                start=(ec == 0), stop=(ec == 7),
                            )
                    nc.scalar.add(dst[:, fc, :], acc[:], b_sb[:, fc:fc + 1])

            # ---- v projection into ones-augmented layout [s, h*65+d] ----
            wv_sb = pp.tile([128, 8, F], MMDT, tag="wv")
            nc.sync.dma_start(wv_sb[:], wvT[:].rearrange("(c p) f -> p c f", p=128))
            bv_row = pp.tile([1, F], DT.float32, tag="bvrow")
            nc.sync.dma_start(bv_row[:], bv[:])
            bv_bc = pp.tile([128, F], DT.float32, tag="bvbc")
            nc.gpsimd.partition_broadcast(bv_bc[:], bv_row[:])
            xv_sb = xp.tile([128, 8, S], MMDT, tag="x")
            nc.sync.dma_start(xv_sb[:], xvT[:].rearrange("(c p) s -> p c s", p=128))
            v_sb = pp.tile([128, 8, HL * 65], MMDT, tag="v")
            for sc in range(8):
                acc = ps.tile([128, F], DT.float32, tag="mm")
                for ec in range(8):
                    nc.tensor.matmul(
                        acc[:],
                        xv_sb[:, ec, sc * 128:(sc + 1) * 128],
                        wv_sb[:, ec, :],
                        start=(ec == 0), stop=(ec == 7),
                    )
                vv = v_sb[:, sc, :].rearrange("p (h c) -> p h c", c=65)
                nc.vector.tensor_add(
                    vv[:, :, 0:64],
                    acc[:].rearrange("p (h d) -> p h d", d=64),
                    bv_bc[:].rearrange("p (h d) -> p h d", d=64),
                )
                nc.vector.memset(vv[:, :, 64:65], 1.0)

            # ---- exp(biasT) from host ----
            expb_sb = pp.tile([128, 8, T], MMDT, tag="expb")
            nc.sync.dma_start(expb_sb[:], expbT[:].rearrange("(c p) t -> p c t", p=128))

            wo_sb = pp.tile([128, 4, E], MMDT, tag="wo")
            nc.sync.dma_start(wo_sb[:], woT[:].rearrange("(c p) e -> p c e", p=128))

            # ---- attention: QK(h) interleaved with PV(h-1) MMs ----
            oT_sb = pp.tile([128, 4, T], MMDT, tag="oT")
            state = {}

            def emit_qk_chunk(h, sc):
                hc, po = h // 2, 64 * (h % 2)
                if sc == 0:
                    pT_new = pTp.tile([128, 8, T], MMDT, tag="pT")
                    state[h] = pT_new
                pT = state[h]
                sps = ps.tile([128, T], DT.float32, tag="mm")
                for th in range(2):
                    nc.tensor.matmul(
                        sps[:, th * 512:(th + 1) * 512],
                        kT_sb[po:po + 64, hc, sc * 128:(sc + 1) * 128],
                        qT_sb[po:po + 64, hc, th * 512:(th + 1) * 512],
                        start=True, stop=True,
                    )
                et = wk.tile([128, T], MMDT, tag="exps")
                nc.scalar.activation(et[:], sps[:],
                                     mybir.ActivationFunctionType.Exp)
                nc.vector.tensor_mul(pT[:, sc, :], et[:], expb_sb[:, sc, :])

            def emit_pv_chunk(h, i):
                # i in 0..7 -> PV matmuls 2i, 2i+1 of head h (16 total)
                pT = state[h]
                if i == 0:
                    ops_new = pspv.tile([65, T], DT.float32, tag="pv")
                    state[(h, "ops")] = ops_new
                ops = state[(h, "ops")]
                for j in (2 * i, 2 * i + 1):
                    th, sc = divmod(j, 8)
                    nc.tensor.matmul(
                        ops[:, th * 512:(th + 1) * 512],
                        v_sb[:, sc, h * 65:(h + 1) * 65],
                        pT[:, sc, th * 512:(th + 1) * 512],
                        start=(sc == 0), stop=(sc == 7),
                    )

            def emit_norm(h):
                hc, po = h // 2, 64 * (h % 2)
                ops = state.pop((h, "ops"))
                del state[h]
                lrow = wk.tile([1, T], DT.float32, tag="lrow")
                nc.vector.tensor_copy(lrow[:], ops[64:65, :])
                rl = wk.tile([1, T], DT.float32, tag="rl")
                nc.vector.reciprocal_approx_fast(out=rl[:], in_=lrow[:])
                rlb = wk.tile([64, T], DT.float32, tag="rlb")
                nc.gpsimd.partition_broadcast(rlb[:], rl[:])
                nc.vector.tensor_mul(oT_sb[po:po + 64, hc, :], ops[0:64, :], rlb[:])

            # heads-0..3 half of the output projection, run during heads 5-6
            # and shipped out as a separate partial (host sums the halves).
            def emit_y01_chunk(slot):
                # slot 0..15 -> (ec8, th) pair; 2 MMs (fc 0,1) + copy on th=1
                ec8, th = divmod(slot, 2)
                if th == 0:
                    yps_new = ps.tile([128, T], DT.float32, tag="mm")
                    state[("y", ec8)] = yps_new
                yps = state[("y", ec8)]
                for fc in range(2):
                    nc.tensor.matmul(
                        yps[:, th * 512:(th + 1) * 512],
                        wo_sb[:, fc, ec8 * 128:(ec8 + 1) * 128],
                        oT_sb[:, fc, th * 512:(th + 1) * 512],
                        start=(fc == 0), stop=(fc == 1),
                    )
                if th == 1:
                    ya = wk.tile([128, T], DT.float32, tag="yout")
                    nc.vector.tensor_copy(ya[:], state.pop(("y", ec8))[:])
                    nc.sync.dma_start(yTa[ec8 * 128:(ec8 + 1) * 128, :], ya[:])

            for sc in range(8):
                emit_qk_chunk(0, sc)
            for h in range(1, HL):
                for sc in range(8):
                    emit_qk_chunk(h, sc)
                    emit_pv_chunk(h - 1, sc)
                    if h in (5, 6):
                        emit_y01_chunk((h - 5) * 8 + sc)
                emit_norm(h - 1)
            for i in range(8):
                emit_pv_chunk(HL - 1, i)
            emit_norm(HL - 1)

            # ---- tail: heads 4..7 half of the output projection ----
            for ec8 in range(8):
                yps = ps.tile([128, T], DT.float32, tag="mm")
                for th in range(2):
                    for fc in range(2, 4):
                        nc.tensor.matmul(
                            yps[:, th * 512:(th + 1) * 512],
                            wo_sb[:, fc, ec8 * 128:(ec8 + 1) * 128],
                            oT_sb[:, fc, th * 512:(th + 1) * 512],
                            start=(fc == 2), stop=(fc == 3),
                        )
                yout = wk.tile([128, T], DT.float32, tag="yout")
                nc.vector.tensor_copy(yout[:], yps[:])
                nc.sync.dma_start(yT[ec8 * 128:(ec8 + 1) * 128, :], yout[:])

    nc.compile()
    return nc


_NC_CACHE = []


def kernel(query, key_, value, edge_bias, attn_mask, key_padding_mask,
           Wq, bq, Wk, bk, Wv, bv, Wo, bo):
    if not _NC_CACHE:
        _NC_CACHE.append(_build_program())
    nc = _NC_CACHE[0]

    scale = np.float32(D ** -0.5)
    q32, k32, v32 = (np.asarray(a, np.float32) for a in (query, key_, value))
    WqT = (np.asarray(Wq, np.float32).T * scale).astype(NP_MMDT)
    WkT = np.asarray(Wk, np.float32).T.astype(NP_MMDT)
    WvT = np.asarray(Wv, np.float32).T.astype(NP_MMDT)
    WoT = np.asarray(Wo, np.float32).T
    bq_s = (np.asarray(bq, np.float32) * scale)
    kpm_add = np.where(np.asarray(key_padding_mask), np.float32(-1e30),
                       np.float32(0.0))  # [B, S]
    mask32 = np.asarray(attn_mask, np.float32)

    in_maps = []
    for c in range(N_CORES):
        b, g = divmod(c, 2)
        cols = slice(g * F, (g + 1) * F)
        bias_sb = (mask32 + np.asarray(edge_bias[b], np.float32)
                   + kpm_add[b][None, :])  # [T, S]
        in_maps.append({
            "xqT": np.ascontiguousarray(q32[b].T).astype(NP_MMDT),
            "xkT": np.ascontiguousarray(k32[b].T).astype(NP_MMDT),
            "xvT": np.ascontiguousarray(v32[b].T).astype(NP_MMDT),
            "wqT": np.ascontiguousarray(WqT[:, cols]),
            "wkT": np.ascontiguousarray(WkT[:, cols]),
            "wvT": np.ascontiguousarray(WvT[:, cols]),
            "woT": np.ascontiguousarray(WoT[cols, :]).astype(NP_MMDT),
            "bq": np.ascontiguousarray(bq_s[cols].reshape(4, 128).T),
            "bk": np.ascontiguousarray(np.asarray(bk, np.float32)[cols]
                                       .reshape(4, 128).T),
            "bv": np.asarray(bv, np.float32)[cols].reshape(1, F),
            "expbT": np.exp(bias_sb.T).astype(NP_MMDT),
        })

    res = run_bass_kernel_spmd(nc, in_maps, list(range(N_CORES)))

    out = np.empty((B, T, E), np.float32)
    bo32 = np.asarray(bo, np.float32)
    for b in range(B):
        r0, r1 = res.results[2 * b], res.results[2 * b + 1]
        acc = r0["yT"] + r0["yTa"] + r1["yT"] + r1["yTa"]
        out[b] = acc.T + bo32[None, :]
    return out


# revision 12
# speedup vs baseline: 1.1362x; 1.0232x over previous
"""Graphormer multi-head attention on 8 Trainium2 cores.

Sharding: 2 cores per batch element (B=4), each core handling 8 of 16 heads
(tensor-parallel within the batch). Per core:
  - QKV projections for its 512 local feature columns (transposed layouts)
  - scoresT[s,t] = K_h Q_h^T per head (K=64 contraction on PE)
  - p = exp(scoresT) * expbT  (expbT = exp(attn_mask + edge_bias).T from host;
    no max-subtraction needed: |scores| stays < ~8)
  - PV with a ones-column appended to V -> row 64 of PSUM = softmax denom
  - normalize via reciprocal + partition-broadcast, out-project 512 local
    features into a full [E,T] partial, summed with the pair core on host.
PV(h-1) matmuls are interleaved into QK(h)'s loop to keep the PE dense
(HAM stays un-throttled). All matmuls run bf16 with fp32 PSUM accumulation.
"""
import sys

sys.path.insert(0, '/opt/trn_rl_repo')

import ml_dtypes
import numpy as np

import concourse.bass as bass
import concourse.mybir as mybir
import concourse.tile as tile
from concourse import bacc
from concourse.bass_utils import run_bass_kernel_spmd

DT = mybir.dt

B, T, S, E, H = 4, 1024, 1024, 1024, 16
D = E // H          # 64
HL = 8              # heads per core
F = HL * D          # 512 local features
N_CORES = 8

MMDT = DT.bfloat16          # matmul operand dtype
NP_MMDT = ml_dtypes.bfloat16


def _build_program():
    nc = bacc.Bacc()

    xqT = nc.dram_tensor("xqT", [E, T], MMDT, kind="ExternalInput")
    xkT = nc.dram_tensor("xkT", [E, S], MMDT, kind="ExternalInput")
    xvT = nc.dram_tensor("xvT", [E, S], MMDT, kind="ExternalInput")
    wqT = nc.dram_tensor("wqT", [E, F], MMDT, kind="ExternalInput")
    wkT = nc.dram_tensor("wkT", [E, F], MMDT, kind="ExternalInput")
    wvT = nc.dram_tensor("wvT", [E, F], MMDT, kind="ExternalInput")
    woT = nc.dram_tensor("woT", [F, E], MMDT, kind="ExternalInput")
    bq = nc.dram_tensor("bq", [128, 4], DT.float32, kind="ExternalInput")
    bk = nc.dram_tensor("bk", [128, 4], DT.float32, kind="ExternalInput")
    bv = nc.dram_tensor("bv", [1, F], DT.float32, kind="ExternalInput")
    expbT = nc.dram_tensor("expbT", [S, T], MMDT, kind="ExternalInput")
    yT = nc.dram_tensor("yT", [E, T], DT.float32, kind="ExternalOutput")
    yTa = nc.dram_tensor("yTa", [E, T], DT.float32, kind="ExternalOutput")

    with tile.TileContext(nc) as tc:
        with tc.tile_pool(name="persist", bufs=1) as pp, \
             tc.tile_pool(name="xin", bufs=2) as xp, \
             tc.tile_pool(name="work", bufs=3) as wk, \
             tc.tile_pool(name="pT", bufs=2) as pTp, \
             tc.tile_pool(name="ps", bufs=2, space="PSUM") as ps, \
             tc.tile_pool(name="pspv", bufs=2, space="PSUM") as pspv:

            # ---- q projection inputs first (critical path) ----
            wq_sb = pp.tile([128, 8, F], MMDT, tag="wq")
            nc.sync.dma_start(wq_sb[:], wqT[:].rearrange("(c p) f -> p c f", p=128))
            bq_sb = pp.tile([128, 4], DT.float32, tag="bq")
            nc.sync.dma_start(bq_sb[:], bq[:])
            qT_sb = pp.tile([128, 4, T], MMDT, tag="qT")
            kT_sb = pp.tile([128, 4, S], MMDT, tag="kT")
            xq_sb = xp.tile([128, 8, T], MMDT, tag="x")
            xqT_v = xqT[:].rearrange("(c p) t -> p c t", p=128)
            for ec in range(8):
                nc.sync.dma_start(xq_sb[:, ec, :], xqT_v[:, ec, :])

            wk_sb = pp.tile([128, 8, F], MMDT, tag="wk")
            nc.sync.dma_start(wk_sb[:], wkT[:].rearrange("(c p) f -> p c f", p=128))
            bk_sb = pp.tile([128, 4], DT.float32, tag="bk")
            nc.sync.dma_start(bk_sb[:], bk[:])
            xk_sb = xp.tile([128, 8, S], MMDT, tag="x")
            nc.sync.dma_start(xk_sb[:], xkT[:].rearrange("(c p) t -> p c t", p=128))

            for x_sb, w_sb, b_sb, dst in (
                (xq_sb, wq_sb, bq_sb, qT_sb),
                (xk_sb, wk_sb, bk_sb, kT_sb),
            ):
                for fc in range(4):
                    acc = ps.tile([128, T], DT.float32, tag="mm")
                    for th in range(2):
                        for ec in range(8):
                            nc.tensor.matmul(
                                acc[:, th * 512:(th + 1) * 512],
                                w_sb[:, ec, fc * 128:(fc + 1) * 128],
                                x_sb[:, ec, th * 512:(th + 1) * 512],
                                start=(ec == 0), stop=(ec == 7),
                            )
                    nc.scalar.add(dst[:, fc, :], acc[:], b_sb[:, fc:fc + 1])

            # ---- v projection into ones-augmented layout [s, h*65+d] ----
            wv_sb = pp.tile([128, 8, F], MMDT, tag="wv")
            nc.sync.dma_start(wv_sb[:], wvT[:].rearrange("(c p) f -> p c f", p=128))
            bv_row = pp.tile([1, F], DT.float32, tag="bvrow")
            nc.sync.dma_start(bv_row[:], bv[:])
            bv_bc = pp.tile([128, F], DT.float32, tag="bvbc")
            nc.gpsimd.partition_broadcast(bv_bc[:], bv_row[:])
            xv_sb = xp.tile([128, 8, S], MMDT, tag="x")
            nc.sync.dma_start(xv_sb[:], xvT[:].rearrange("(c p) s -> p c s", p=128))
            v_sb = pp.tile([128, 8, HL * 65], MMDT, tag="v")
            for sc in range(8):
                acc = ps.tile([128, F], DT.float32, tag="mm")
                for ec in range(8):
                    nc.tensor.matmul(
                        acc[:],
                        xv_sb[:, ec, sc * 128:(sc + 1) * 128],
                        wv_sb[:, ec, :],
                        start=(ec == 0), stop=(ec == 7),
                    )
                vv = v_sb[:, sc, :].rearrange("p (h c) -> p h c", c=65)
                nc.vector.tensor_add(
                    vv[:, :, 0:64],
                    acc[:].rearrange("p (h d) -> p h d", d=64),
                    bv_bc[:].rearrange("p (h d) -> p h d", d=64),
                )
                nc.vector.memset(vv[:, :, 64:65], 1.0)

            # ---- exp(biasT) from host ----
            expb_sb = pp.tile([128, 8, T], MMDT, tag="expb")
            nc.sync.dma_start(expb_sb[:], expbT[:].rearrange("(c p) t -> p c t", p=128))

            wo_sb = pp.tile([128, 4, E], MMDT, tag="wo")
            nc.sync.dma_start(wo_sb[:], woT[:].rearrange("(c p) e -> p c e", p=128))

            # ---- attention: QK(h) interleaved with PV(h-1) MMs ----
            oT_sb = pp.tile([128, 4, T], MMDT, tag="oT")
            state = {}

            def emit_qk_chunk(h, sc):
                hc, po = h // 2, 64 * (h % 2)
                if sc == 0:
                    pT_new = pTp.tile([128, 8, T], MMDT, tag="pT")
                    state[h] = pT_new
                pT = state[h]
                sps = ps.tile([128, T], DT.float32, tag="mm")
                for th in range(2):
                    nc.tensor.matmul(
                        sps[:, th * 512:(th + 1) * 512],
                        kT_sb[po:po + 64, hc, sc * 128:(sc + 1) * 128],
                        qT_sb[po:po + 64, hc, th * 512:(th + 1) * 512],
                        start=True, stop=True,
                    )
                et = wk.tile([128, T], MMDT, tag="exps")
                nc.scalar.activation(et[:], sps[:],
                                     mybir.ActivationFunctionType.Exp)
                nc.vector.tensor_mul(pT[:, sc, :], et[:], expb_sb[:, sc, :])

            def emit_pv_chunk(h, i):
                # i in 0..7 -> PV matmuls 2i, 2i+1 of head h (16 total)
                pT = state[h]
                if i == 0:
                    ops_new = pspv.tile([65, T], DT.float32, tag="pv")
                    state[(h, "ops")] = ops_new
                ops = state[(h, "ops")]
                for j in (2 * i, 2 * i + 1):
                    th, sc = divmod(j, 8)
                    nc.tensor.matmul(
                        ops[:, th * 512:(th + 1) * 512],
                        v_sb[:, sc, h * 65:(h + 1) * 65],
                        pT[:, sc, th * 512:(th + 1) * 512],
                        start=(sc == 0), stop=(sc == 7),
                    )

            def emit_norm(h):
                hc, po = h // 2, 64 * (h % 2)
                ops = state.pop((h, "ops"))
                del state[h]
                lrow = wk.tile([1, T], DT.float32, tag="lrow")
                nc.vector.tensor_copy(lrow[:], ops[64:65, :])
                rl = wk.tile([1, T], DT.float32, tag="rl")
                nc.vector.reciprocal_approx_fast(out=rl[:], in_=lrow[:])
                rlb = wk.tile([64, T], DT.float32, tag="rlb")
                nc.gpsimd.partition_broadcast(rlb[:], rl[:])
                nc.vector.tensor_mul(oT_sb[po:po + 64, hc, :], ops[0:64, :], rlb[:])

            # heads-0..3 half of the output projection, run during heads 5-6
            # and shipped out as a separate partial (host sums the halves).
            def emit_y01_chunk(slot):
                # slot 0..15 -> (ec8, th) pair; 2 MMs (fc 0,1) + copy on th=1
                ec8, th = divmod(slot, 2)
                if th == 0:
                    yps_new = ps.tile([128, T], DT.float32, tag="mm")
                    state[("y", ec8)] = yps_new
                yps = state[("y", ec8)]
                for fc in range(2):
                    nc.tensor.matmul(
                        yps[:, th * 512:(th + 1) * 512],
                        wo_sb[:, fc, ec8 * 128:(ec8 + 1) * 128],
                        oT_sb[:, fc, th * 512:(th + 1) * 512],
                        start=(fc == 0), stop=(fc == 1),
                    )
                if th == 1:
                    ya = wk.tile([128, T], DT.float32, tag="yout")
                    nc.vector.tensor_copy(ya[:], state.pop(("y", ec8))[:])
                    nc.sync.dma_start(yTa[ec8 * 128:(ec8 + 1) * 128, :], ya[:])

            for sc in range(8):
                emit_qk_chunk(0, sc)
            for h in range(1, HL):
                for sc in range(8):
                    emit_qk_chunk(h, sc)
                    emit_pv_chunk(h - 1, sc)
                    if h in (5, 6):
                        emit_y01_chunk((h - 5) * 8 + sc)
                emit_norm(h - 1)
            for i in range(8):
                emit_pv_chunk(HL - 1, i)
            emit_norm(HL - 1)

            # ---- tail: heads 4..7 half of the output projection ----
            for ec8 in range(8):
                yps = ps.tile([128, T], DT.float32, tag="mm")
                for th in range(2):
                    for fc in range(2, 4):
                        nc.tensor.matmul(
                            yps[:, th * 512:(th + 1) * 512],
                            wo_sb[:, fc, ec8 * 128:(ec8 + 1) * 128],
                            oT_sb[:, fc, th * 512:(th + 1) * 512],
                            start=(fc == 2), stop=(fc == 3),
                        )
                yout = wk.tile([128, T], DT.float32, tag="yout")
                nc.vector.tensor_copy(yout[:], yps[:])
                nc.sync.dma_start(yT[ec8 * 128:(ec8 + 1) * 128, :], yout[:])

    nc.compile()
    return nc


_NC_CACHE = []


def kernel(query, key_, value, edge_bias, attn_mask, key_padding_mask,
           Wq, bq, Wk, bk, Wv, bv, Wo, bo):
    if not _NC_CACHE:
        _NC_CACHE.append(_build_program())
    nc = _NC_CACHE[0]

    scale = np.float32(D ** -0.5)
    q32, k32, v32 = (np.asarray(a, np.float32) for a in (query, key_, value))
    WqT = (np.asarray(Wq, np.float32).T * scale).astype(NP_MMDT)
    WkT = np.asarray(Wk, np.float32).T.astype(NP_MMDT)
    WvT = np.asarray(Wv, np.float32).T.astype(NP_MMDT)
    WoT = np.asarray(Wo, np.float32).T
    bq_s = (np.asarray(bq, np.float32) * scale)
    kpm_add = np.where(np.asarray(key_padding_mask), np.float32(-1e30),
                       np.float32(0.0))  # [B, S]
    mask32 = np.asarray(attn_mask, np.float32)

    in_maps = []
    for c in range(N_CORES):
        b, g = divmod(c, 2)
        cols = slice(g * F, (g + 1) * F)
        bias_sb = (mask32 + np.asarray(edge_bias[b], np.float32)
                   + kpm_add[b][None, :])  # [T, S]
        in_maps.append({
            "xqT": np.ascontiguousarray(q32[b].T).astype(NP_MMDT),
            "xkT": np.ascontiguousarray(k32[b].T).astype(NP_MMDT),
            "xvT": np.ascontiguousarray(v32[b].T).astype(NP_MMDT),
            "wqT": np.ascontiguousarray(WqT[:, cols]),
            "wkT": np.ascontiguousarray(WkT[:, cols]),
            "wvT": np.ascontiguousarray(WvT[:, cols]),
            "woT": np.ascontiguousarray(WoT[cols, :]).astype(NP_MMDT),
            "bq": np.ascontiguousarray(bq_s[cols].reshape(4, 128).T),
            "bk": np.ascontiguousarray(np.asarray(bk, np.float32)[cols]
                                       .reshape(4, 128).T),
            "bv": np.asarray(bv, np.float32)[cols].reshape(1, F),
            "expbT": np.exp(bias_sb.T).astype(NP_MMDT),
        })

    res = run_bass_kernel_spmd(nc, in_maps, list(range(N_CORES)))

    out = np.empty((B, T, E), np.float32)
    bo32 = np.asarray(bo, np.float32)
    for b in range(B):
        r0, r1 = res.results[2 * b], res.results[2 * b + 1]
        acc = r0["yT"] + r0["yTa"] + r1["yT"] + r1["yTa"]
        out[b] = acc.T + bo32[None, :]
    return out
